# revision 1
# baseline (speedup 1.0000x reference)
"""GSA (channel/XCA attention) kernel for Trainium2, data-parallel over batch.

Full inputs in, full outputs out. Batch 16 is split 2-per-core across the
8 NeuronCores via jax.pmap; every core runs the whole block (qkv 1x1 conv,
depthwise 3x3, l2-normalized channel attention, proj 1x1 conv) on its batch
shard. No collectives needed. Shapes hardcoded per the problem spec:
x (16, 512, 64, 64) f32.
"""
import numpy as np
import jax
import jax.numpy as jnp

C = 512
HEADS = 8
B, H, W = 16, 64, 64
N_CORES = 8


def _l2norm(t):
    n = jnp.sqrt(jnp.sum(t * t, axis=-1, keepdims=True))
    return t / jnp.maximum(n, 1e-12)


def _forward(x, qkv_w, dw_w, proj_w, temperature):
    b, c, h, w = x.shape
    hd = c // HEADS
    # 1x1 conv as a matmul over channels
    qkv = jnp.einsum('oc,bchw->bohw', qkv_w, x)
    # depthwise 3x3 SAME conv via 9 shifted multiply-adds (avoids grouped-conv
    # lowering; feature_group_count=1536 is the slow path on this backend)
    p = jnp.pad(qkv, ((0, 0), (0, 0), (1, 1), (1, 1)))
    acc = jnp.zeros_like(qkv)
    for di in range(3):
        for dj in range(3):
            acc = acc + p[:, :, di:di + h, dj:dj + w] * dw_w[:, 0, di, dj][None, :, None, None]
    qkv = acc
    q, k, v = jnp.split(qkv, 3, axis=1)

    def to_heads(t):
        return t.reshape(b, HEADS, hd, h * w)

    q, k, v = to_heads(q), to_heads(k), to_heads(v)
    q = _l2norm(q)
    k = _l2norm(k)
    attn = jnp.einsum('bhcn,bhdn->bhcd', q, k) * temperature
    attn = jax.nn.relu(attn)
    out = jnp.einsum('bhcd,bhdn->bhcn', attn, v)
    y = out.reshape(b, c, h, w)
    y = jnp.einsum('oc,bchw->bohw', proj_w, y)
    return y, attn


_pmapped = None


def _get_pmapped():
    global _pmapped
    if _pmapped is None:
        devs = jax.devices()[:N_CORES]
        _pmapped = jax.pmap(
            _forward, in_axes=(0, None, None, None, None), devices=devs)
    return _pmapped

def kernel(x, qkv_w, dw_w, proj_w, temperature):
    per = B // N_CORES
    xs = np.ascontiguousarray(x.reshape(N_CORES, per, C, H, W))
    fn = _get_pmapped()
    y, attn = fn(xs, jnp.asarray(qkv_w), jnp.asarray(dw_w),
                 jnp.asarray(proj_w), jnp.asarray(temperature))
    y = np.asarray(y).reshape(B, C, H, W)
    attn = np.asarray(attn).reshape(B, HEADS, C // HEADS, C // HEADS)
    return y, attn
